# revision 2
# baseline (speedup 1.0000x reference)
"""ContextAttention (Bahdanau-style additive attention pooling) on 8 trn2 cores.

Reference math (N=M=1024, D=256):
  q = f_r @ W_w.T + W_b                     [N, D]
  k = f_r_prime @ Wp_w.T + Wp_b             [M, D]
  S[n,m]   = sum_d w_w[d] * tanh(q[n,d] + k[m,d])   (+ w_b, cancels in softmax)
  alpha    = softmax_m(S)                   [N, M]
  context  = alpha @ f_r_prime              [N, D]
  alpha_p  = softmax_n(context @ wp_w.T)    (+ wp_b, cancels)
  pool     = alpha_p.T @ context            [1, D]

Sharding: N split across 8 cores (128 rows each); f_r_prime + weights
replicated. Each core returns its context rows and per-row pooling scores;
the final softmax over N + weighted sum (the tiny cross-core reduction) is
done on host after gathering.

Device kernel layout (per core, n = 0..127 local rows):
  - d (=256) lives on partitions in 2 chunks of 128 for the tanh phase, so
    the per-row bias add q_n[d] fuses into the ScalarE ACTIVATE instruction.
  - The weighted reduction over d uses the PE with a zero-embedded stationary
    w_ext[:, 128-n : 256-n]  (w in column n, zeros elsewhere): each matmul
    writes the full [128, 512] PSUM tile but only row n receives nonzero
    values; accumulation over all n/chunks builds the [128, 1024] score tile
    exactly (zero adds are exact in fp32).
"""

import sys

sys.path.insert(0, "/opt/trn_rl_repo")

import numpy as np

import concourse.bacc as bacc
import concourse.bass as bass
import concourse.mybir as mybir
from concourse import tile
from concourse.bass_utils import run_bass_kernel_spmd

N, M, D = 1024, 1024, 256
N_CORES = 8
NP = N // N_CORES  # 128 rows per core
P = 128  # partitions
DT = mybir.dt.float32
F32 = np.float32

_CACHE = {}


def build_nc():
    nc = bacc.Bacc("TRN2", target_bir_lowering=False, debug=False, num_devices=N_CORES)

    # ---- DRAM parameters (per-core shapes) ----
    frT = nc.declare_dram_parameter("frT", [D, NP], DT, isOutput=False)
    fpT = nc.declare_dram_parameter("fpT", [D, M], DT, isOutput=False)
    fp = nc.declare_dram_parameter("fp", [M, D], DT, isOutput=False)
    WwT = nc.declare_dram_parameter("WwT", [D, D], DT, isOutput=False)
    WpT = nc.declare_dram_parameter("WpT", [D, D], DT, isOutput=False)
    Wb = nc.declare_dram_parameter("Wb", [D, 1], DT, isOutput=False)
    Wpb = nc.declare_dram_parameter("Wpb", [D, 1], DT, isOutput=False)
    wext = nc.declare_dram_parameter("wext", [D, 2 * P], DT, isOutput=False)
    wpB = nc.declare_dram_parameter("wpB", [P, D], DT, isOutput=False)
    ident = nc.declare_dram_parameter("ident", [P, P], DT, isOutput=False)

    ctx_out = nc.declare_dram_parameter("ctx_out", [NP, D], DT, isOutput=True)
    s_out = nc.declare_dram_parameter("s_out", [NP, 1], DT, isOutput=True)

    KC = D // P  # 2 contraction chunks

    with tile.TileContext(nc) as tc:
        with (
            tc.tile_pool(name="const", bufs=1) as cpool,
            tc.tile_pool(name="work", bufs=3) as wpool,
            tc.tile_pool(name="small", bufs=2) as spool,
            tc.tile_pool(name="alpha", bufs=8) as apool,
            tc.tile_pool(name="ps_qk", bufs=2, space="PSUM") as ps_qk,
            tc.tile_pool(name="ps_s", bufs=1, space="PSUM") as ps_s,
            tc.tile_pool(name="ps_tr", bufs=2, space="PSUM") as ps_tr,
            tc.tile_pool(name="ps_ctx", bufs=1, space="PSUM") as ps_ctx,
        ):
            # ---- load constants ----
            frT_sb = []
            WwT_sb = []
            WpT_sb = []
            Wb_sb = []
            Wpb_sb = []
            wext_sb = []
            for k in range(KC):
                t_frT = cpool.tile([P, NP], DT, name=f"frT{k}")
                nc.sync.dma_start(out=t_frT[:, :], in_=frT[k * P : (k + 1) * P, :])
                frT_sb.append(t_frT)
                t_WwT = cpool.tile([P, D], DT, name=f"WwT{k}")
                nc.sync.dma_start(out=t_WwT[:, :], in_=WwT[k * P : (k + 1) * P, :])
                WwT_sb.append(t_WwT)
                t_WpT = cpool.tile([P, D], DT, name=f"WpT{k}")
                nc.sync.dma_start(out=t_WpT[:, :], in_=WpT[k * P : (k + 1) * P, :])
                WpT_sb.append(t_WpT)
                t_Wb = cpool.tile([P, 1], DT, name=f"Wb{k}")
                nc.sync.dma_start(out=t_Wb[:, :], in_=Wb[k * P : (k + 1) * P, :])
                Wb_sb.append(t_Wb)
                t_Wpb = cpool.tile([P, 1], DT, name=f"Wpb{k}")
                nc.sync.dma_start(out=t_Wpb[:, :], in_=Wpb[k * P : (k + 1) * P, :])
                Wpb_sb.append(t_Wpb)
                t_wx = cpool.tile([P, 2 * P], DT, name=f"wext{k}")
                nc.sync.dma_start(out=t_wx[:, :], in_=wext[k * P : (k + 1) * P, :])
                wext_sb.append(t_wx)
            fpT_sb = []
            for k in range(KC):
                t_fpT = cpool.tile([P, M], DT, name=f"fpT{k}")
                nc.sync.dma_start(out=t_fpT[:, :], in_=fpT[k * P : (k + 1) * P, :])
                fpT_sb.append(t_fpT)
            fp_sb = []
            for j in range(M // P):
                t_fp = cpool.tile([P, D], DT, name=f"fp{j}")
                nc.sync.dma_start(out=t_fp[:, :], in_=fp[j * P : (j + 1) * P, :])
                fp_sb.append(t_fp)
            wpB_sb = cpool.tile([P, D], DT, name="wpB")
            nc.sync.dma_start(out=wpB_sb[:, :], in_=wpB[:, :])
            ident_sb = cpool.tile([P, P], DT, name="ident")
            nc.sync.dma_start(out=ident_sb[:, :], in_=ident[:, :])

            # ---- qT[d', n] = sum_d WwT[d, d'] * frT[d, n]  (+ Wb) ----
            qT_sb = []
            for i in range(KC):
                q_ps = ps_qk.tile([P, 512], DT, name="q_ps", tag="qk")
                for k in range(KC):
                    nc.tensor.matmul(
                        q_ps[:, :NP],
                        lhsT=WwT_sb[k][:, i * P : (i + 1) * P],
                        rhs=frT_sb[k][:, :],
                        start=(k == 0),
                        stop=(k == KC - 1),
                    )
                t_qT = cpool.tile([P, NP], DT, name=f"qT{i}")
                nc.vector.tensor_scalar_add(t_qT[:, :], q_ps[:, :NP], Wb_sb[i][:, 0:1])
                qT_sb.append(t_qT)

            # ---- kT[d, m] = sum_d'' WpT[d'', d] * fpT[d'', m]  (+ Wpb) ----
            kT_sb = []
            for i in range(KC):
                t_kT = cpool.tile([P, M], DT, name=f"kT{i}")
                for h in range(M // 512):
                    k_ps = ps_qk.tile([P, 512], DT, name="k_ps", tag="qk")
                    for k in range(KC):
                        nc.tensor.matmul(
                            k_ps[:, :],
                            lhsT=WpT_sb[k][:, i * P : (i + 1) * P],
                            rhs=fpT_sb[k][:, h * 512 : (h + 1) * 512],
                            start=(k == 0),
                            stop=(k == KC - 1),
                        )
                    nc.vector.tensor_scalar_add(
                        t_kT[:, h * 512 : (h + 1) * 512], k_ps[:, :], Wpb_sb[i][:, 0:1]
                    )
                kT_sb.append(t_kT)

            # ---- phase B: S[n, m] = sum_d w[d] * tanh(kT[d, m] + qT[d, n]) ----
            S_ps = ps_s.tile([P, M], DT, name="S_ps")
            for n in range(NP):
                for ci in range(KC):
                    T = wpool.tile([P, M], DT, name="T")
                    nc.scalar.activation(
                        T[:, :],
                        kT_sb[ci][:, :],
                        mybir.ActivationFunctionType.Tanh,
                        bias=qT_sb[ci][:, n : n + 1],
                    )
                    for h in range(M // 512):
                        nc.tensor.matmul(
                            S_ps[:, h * 512 : (h + 1) * 512],
                            lhsT=wext_sb[ci][:, P - n : 2 * P - n],
                            rhs=T[:, h * 512 : (h + 1) * 512],
                            start=(n == 0 and ci == 0),
                            stop=(n == NP - 1 and ci == KC - 1),
                        )

            # ---- softmax over m (unnormalized; row scale applied to context) ----
            mx = spool.tile([P, 1], DT, name="mx")
            nc.vector.reduce_max(mx[:, :], S_ps[:, :], axis=mybir.AxisListType.X)
            negmx = spool.tile([P, 1], DT, name="negmx")
            nc.vector.tensor_scalar_mul(negmx[:, :], mx[:, :], -1.0)
            expS = wpool.tile([P, M], DT, name="expS", bufs=1)
            sumex = spool.tile([P, 1], DT, name="sumex")
            nc.scalar.activation(
                expS[:, :],
                S_ps[:, :],
                mybir.ActivationFunctionType.Exp,
                bias=negmx[:, 0:1],
                accum_out=sumex[:, 0:1],
            )
            rs = spool.tile([P, 1], DT, name="rs")
            nc.vector.reciprocal(rs[:, :], sumex[:, :])

            # ---- context = alpha @ f_r_prime ----
            ctx_ps = ps_ctx.tile([P, D], DT, name="ctx_ps")
            for j in range(M // P):
                tr_ps = ps_tr.tile([P, P], DT, name="tr_ps")
                nc.tensor.transpose(
                    tr_ps[:, :], expS[:, j * P : (j + 1) * P], ident_sb[:, :]
                )
                aT = apool.tile([P, P], DT, name="aT")
                nc.vector.tensor_copy(aT[:, :], tr_ps[:, :])
                nc.tensor.matmul(
                    ctx_ps[:, :],
                    lhsT=aT[:, :],
                    rhs=fp_sb[j][:, :],
                    start=(j == 0),
                    stop=(j == M // P - 1),
                )
            ctx_sb = wpool.tile([P, D], DT, name="ctx_sb", bufs=1)
            nc.vector.tensor_scalar_mul(ctx_sb[:, :], ctx_ps[:, :], rs[:, 0:1])

            # ---- per-row pooling score s[n] = context[n, :] . wp_w ----
            tmp = wpool.tile([P, D], DT, name="tmp", bufs=1)
            nc.vector.tensor_mul(tmp[:, :], ctx_sb[:, :], wpB_sb[:, :])
            s_sb = spool.tile([P, 1], DT, name="s_sb")
            nc.vector.reduce_sum(s_sb[:, :], tmp[:, :], axis=mybir.AxisListType.X)

            # ---- outputs ----
            nc.sync.dma_start(out=ctx_out[:, :], in_=ctx_sb[:, :])
            nc.sync.dma_start(out=s_out[:, :], in_=s_sb[:, :])

    nc.finalize()
    return nc


def _prep_inputs(f_r, f_r_prime, W_w, W_b, Wp_w, Wp_b, w_w, w_b, wp_w, wp_b):
    """Host-side layout prep (transposes / broadcasts only) + sharding."""
    fpT = np.ascontiguousarray(f_r_prime.T, dtype=F32)
    fp = np.ascontiguousarray(f_r_prime, dtype=F32)
    WwT = np.ascontiguousarray(W_w.T, dtype=F32)
    WpT = np.ascontiguousarray(Wp_w.T, dtype=F32)
    Wb = np.ascontiguousarray(W_b.reshape(D, 1), dtype=F32)
    Wpb = np.ascontiguousarray(Wp_b.reshape(D, 1), dtype=F32)
    # zero-embedded w: column P (=128) holds w, so slice [P-n : 2P-n] puts w
    # in column n of the stationary operand.
    wext = np.zeros((D, 2 * P), dtype=F32)
    wext[:, P] = w_w.reshape(D)
    wpB = np.broadcast_to(wp_w.reshape(1, D), (P, D)).copy()
    ident = np.eye(P, dtype=F32)

    shared = {
        "fpT": fpT,
        "fp": fp,
        "WwT": WwT,
        "WpT": WpT,
        "Wb": Wb,
        "Wpb": Wpb,
        "wext": wext,
        "wpB": wpB,
        "ident": ident,
    }
    in_maps = []
    for c in range(N_CORES):
        frT = np.ascontiguousarray(f_r[c * NP : (c + 1) * NP, :].T, dtype=F32)
        in_maps.append({"frT": frT, **shared})
    return in_maps


def _run(in_maps, **kw):
    if "nc" not in _CACHE:
        _CACHE["nc"] = build_nc()
    return run_bass_kernel_spmd(_CACHE["nc"], in_maps, list(range(N_CORES)), **kw)


def kernel(f_r, f_r_prime, W_w, W_b, Wp_w, Wp_b, w_w, w_b, wp_w, wp_b):
    in_maps = _prep_inputs(
        f_r, f_r_prime, W_w, W_b, Wp_w, Wp_b, w_w, w_b, wp_w, wp_b
    )
    res = _run(in_maps)
    ctx = np.concatenate([res.results[c]["ctx_out"] for c in range(N_CORES)], axis=0)
    s = np.concatenate(
        [res.results[c]["s_out"][:, 0] for c in range(N_CORES)], axis=0
    ).astype(np.float64)
    # final cross-shard softmax over N + pooled sum (the "all-reduce" step)
    s -= s.max()
    e = np.exp(s)
    a = (e / e.sum()).astype(F32)
    pool = a[None, :] @ ctx  # [1, D]
    return pool.astype(F32)


# revision 4
# speedup vs baseline: 1.7271x; 1.7271x over previous
"""ContextAttention (Bahdanau-style additive attention pooling) on 8 trn2 cores.

Reference math (N=M=1024, D=256):
  q = f_r @ W_w.T + W_b                     [N, D]
  k = f_r_prime @ Wp_w.T + Wp_b             [M, D]
  S[n,m]   = sum_d w_w[d] * tanh(q[n,d] + k[m,d])   (+ w_b, cancels in softmax)
  alpha    = softmax_m(S)                   [N, M]
  context  = alpha @ f_r_prime              [N, D]
  alpha_p  = softmax_n(context @ wp_w.T)    (+ wp_b, cancels)
  pool     = alpha_p.T @ context            [1, D]

Sharding: N split across 8 cores (128 rows each); f_r_prime + weights
replicated. Each core returns its context rows and per-row pooling scores;
the final softmax over N + weighted sum (the tiny cross-core reduction) is
done on host after gathering.

Device kernel layout (per core, n = 0..127 local rows):
  - d (=256) lives on partitions in 2 chunks of 128 for the tanh phase, so
    the per-row bias add q_n[d] fuses into the ScalarE ACTIVATE instruction.
  - The weighted reduction over d uses the PE with a zero-embedded stationary
    w_ext[:, 128-n : 256-n]  (w in column n, zeros elsewhere): each matmul
    writes the full [128, 512] PSUM tile but only row n receives nonzero
    values; accumulation over all n/chunks builds the [128, 1024] score tile
    exactly (zero adds are exact in fp32).
"""

import sys

sys.path.insert(0, "/opt/trn_rl_repo")

import numpy as np

import concourse.bacc as bacc
import concourse.bass as bass
import concourse.mybir as mybir
from concourse import tile
from concourse.bass_utils import run_bass_kernel_spmd

N, M, D = 1024, 1024, 256
N_CORES = 8
NP = N // N_CORES  # 128 rows per core
P = 128  # partitions
DT = mybir.dt.float32
BF = mybir.dt.bfloat16
F32 = np.float32

_CACHE = {}


def build_nc():
    nc = bacc.Bacc("TRN2", target_bir_lowering=False, debug=False, num_devices=N_CORES)

    # ---- DRAM parameters (per-core shapes) ----
    frT = nc.declare_dram_parameter("frT", [D, NP], DT, isOutput=False)
    fpT = nc.declare_dram_parameter("fpT", [D, M], DT, isOutput=False)
    fp = nc.declare_dram_parameter("fp", [M, D], DT, isOutput=False)
    WwT = nc.declare_dram_parameter("WwT", [D, D], DT, isOutput=False)
    WpT = nc.declare_dram_parameter("WpT", [D, D], DT, isOutput=False)
    Wb = nc.declare_dram_parameter("Wb", [D, 1], DT, isOutput=False)
    Wpb = nc.declare_dram_parameter("Wpb", [D, 1], DT, isOutput=False)
    wext = nc.declare_dram_parameter("wext", [D, 2 * P], BF, isOutput=False)
    wpB = nc.declare_dram_parameter("wpB", [P, D], DT, isOutput=False)
    ident = nc.declare_dram_parameter("ident", [P, P], DT, isOutput=False)

    ctx_out = nc.declare_dram_parameter("ctx_out", [NP, D], DT, isOutput=True)
    s_out = nc.declare_dram_parameter("s_out", [NP, 1], DT, isOutput=True)

    KC = D // P  # 2 contraction chunks

    with tile.TileContext(nc) as tc:
        with (
            tc.tile_pool(name="const", bufs=1) as cpool,
            tc.tile_pool(name="work", bufs=3) as wpool,
            tc.tile_pool(name="small", bufs=2) as spool,
            tc.tile_pool(name="alpha", bufs=8) as apool,
            tc.tile_pool(name="ps_qk", bufs=2, space="PSUM") as ps_qk,
            tc.tile_pool(name="ps_s", bufs=1, space="PSUM") as ps_s,
            tc.tile_pool(name="ps_tr", bufs=2, space="PSUM") as ps_tr,
            tc.tile_pool(name="ps_ctx", bufs=1, space="PSUM") as ps_ctx,
        ):
            # ---- load constants ----
            frT_sb = []
            WwT_sb = []
            WpT_sb = []
            Wb_sb = []
            Wpb_sb = []
            wext_sb = []
            for k in range(KC):
                t_frT = cpool.tile([P, NP], DT, name=f"frT{k}")
                nc.sync.dma_start(out=t_frT[:, :], in_=frT[k * P : (k + 1) * P, :])
                frT_sb.append(t_frT)
                t_WwT = cpool.tile([P, D], DT, name=f"WwT{k}")
                nc.sync.dma_start(out=t_WwT[:, :], in_=WwT[k * P : (k + 1) * P, :])
                WwT_sb.append(t_WwT)
                t_WpT = cpool.tile([P, D], DT, name=f"WpT{k}")
                nc.sync.dma_start(out=t_WpT[:, :], in_=WpT[k * P : (k + 1) * P, :])
                WpT_sb.append(t_WpT)
                t_Wb = cpool.tile([P, 1], DT, name=f"Wb{k}")
                nc.sync.dma_start(out=t_Wb[:, :], in_=Wb[k * P : (k + 1) * P, :])
                Wb_sb.append(t_Wb)
                t_Wpb = cpool.tile([P, 1], DT, name=f"Wpb{k}")
                nc.sync.dma_start(out=t_Wpb[:, :], in_=Wpb[k * P : (k + 1) * P, :])
                Wpb_sb.append(t_Wpb)
                t_wx = cpool.tile([P, 2 * P], BF, name=f"wext{k}")
                nc.sync.dma_start(out=t_wx[:, :], in_=wext[k * P : (k + 1) * P, :])
                wext_sb.append(t_wx)
            fpT_sb = []
            for k in range(KC):
                t_fpT = cpool.tile([P, M], DT, name=f"fpT{k}")
                nc.sync.dma_start(out=t_fpT[:, :], in_=fpT[k * P : (k + 1) * P, :])
                fpT_sb.append(t_fpT)
            fp_sb = []
            for j in range(M // P):
                t_fp = cpool.tile([P, D], DT, name=f"fp{j}")
                nc.sync.dma_start(out=t_fp[:, :], in_=fp[j * P : (j + 1) * P, :])
                fp_sb.append(t_fp)
            wpB_sb = cpool.tile([P, D], DT, name="wpB")
            nc.sync.dma_start(out=wpB_sb[:, :], in_=wpB[:, :])
            ident_sb = cpool.tile([P, P], DT, name="ident")
            nc.sync.dma_start(out=ident_sb[:, :], in_=ident[:, :])

            # ---- qT[d', n] = sum_d WwT[d, d'] * frT[d, n]  (+ Wb) ----
            qT_sb = []
            for i in range(KC):
                q_ps = ps_qk.tile([P, 512], DT, name="q_ps", tag="qk")
                for k in range(KC):
                    nc.tensor.matmul(
                        q_ps[:, :NP],
                        lhsT=WwT_sb[k][:, i * P : (i + 1) * P],
                        rhs=frT_sb[k][:, :],
                        start=(k == 0),
                        stop=(k == KC - 1),
                    )
                t_qT = cpool.tile([P, NP], DT, name=f"qT{i}")
                nc.vector.tensor_scalar_add(t_qT[:, :], q_ps[:, :NP], Wb_sb[i][:, 0:1])
                qT_sb.append(t_qT)

            # ---- kT[d, m] = sum_d'' WpT[d'', d] * fpT[d'', m]  (+ Wpb) ----
            kT_sb = []
            for i in range(KC):
                t_kT = cpool.tile([P, M], DT, name=f"kT{i}")
                for h in range(M // 512):
                    k_ps = ps_qk.tile([P, 512], DT, name="k_ps", tag="qk")
                    for k in range(KC):
                        nc.tensor.matmul(
                            k_ps[:, :],
                            lhsT=WpT_sb[k][:, i * P : (i + 1) * P],
                            rhs=fpT_sb[k][:, h * 512 : (h + 1) * 512],
                            start=(k == 0),
                            stop=(k == KC - 1),
                        )
                    nc.vector.tensor_scalar_add(
                        t_kT[:, h * 512 : (h + 1) * 512], k_ps[:, :], Wpb_sb[i][:, 0:1]
                    )
                kT_sb.append(t_kT)

            # ---- phase B: S[n, m] = sum_d w[d] * tanh(kT[d, m] + qT[d, n]) ----
            # (n, chunk) pairs are processed in groups of B; the per-row bias
            # add runs on DVE (tensor_scalar, 2x fp32), tanh runs on ScalarE
            # over the whole [P, B*M] group tile (amortizes the per-instr
            # overhead), and the PE reduces with bf16 inputs at full rate.
            B = 8  # (n, chunk) pairs per ACT instruction
            NITEMS = NP * KC
            S_ps = ps_s.tile([P, M], DT, name="S_ps")
            for g in range(NITEMS // B):
                Tin = wpool.tile([P, B * M], DT, name="Tin", bufs=2)
                for j in range(B):
                    idx = g * B + j
                    n, ci = idx // KC, idx % KC
                    nc.vector.tensor_scalar_add(
                        Tin[:, j * M : (j + 1) * M],
                        kT_sb[ci][:, :],
                        qT_sb[ci][:, n : n + 1],
                    )
                Tout = wpool.tile([P, B * M], BF, name="Tout", bufs=2)
                nc.scalar.activation(
                    Tout[:, :], Tin[:, :], mybir.ActivationFunctionType.Tanh
                )
                for j in range(B):
                    idx = g * B + j
                    n, ci = idx // KC, idx % KC
                    for h in range(M // 512):
                        nc.tensor.matmul(
                            S_ps[:, h * 512 : (h + 1) * 512],
                            lhsT=wext_sb[ci][:, P - n : 2 * P - n],
                            rhs=Tout[:, j * M + h * 512 : j * M + (h + 1) * 512],
                            start=(idx == 0),
                            stop=(idx == NITEMS - 1),
                        )

            # ---- softmax over m (unnormalized; row scale applied to context) ----
            mx = spool.tile([P, 1], DT, name="mx")
            nc.vector.reduce_max(mx[:, :], S_ps[:, :], axis=mybir.AxisListType.X)
            negmx = spool.tile([P, 1], DT, name="negmx")
            nc.vector.tensor_scalar_mul(negmx[:, :], mx[:, :], -1.0)
            expS = wpool.tile([P, M], DT, name="expS", bufs=1)
            sumex = spool.tile([P, 1], DT, name="sumex")
            nc.scalar.activation(
                expS[:, :],
                S_ps[:, :],
                mybir.ActivationFunctionType.Exp,
                bias=negmx[:, 0:1],
                accum_out=sumex[:, 0:1],
            )
            rs = spool.tile([P, 1], DT, name="rs")
            nc.vector.reciprocal(rs[:, :], sumex[:, :])

            # ---- context = alpha @ f_r_prime ----
            ctx_ps = ps_ctx.tile([P, D], DT, name="ctx_ps")
            for j in range(M // P):
                tr_ps = ps_tr.tile([P, P], DT, name="tr_ps")
                nc.tensor.transpose(
                    tr_ps[:, :], expS[:, j * P : (j + 1) * P], ident_sb[:, :]
                )
                aT = apool.tile([P, P], DT, name="aT")
                nc.vector.tensor_copy(aT[:, :], tr_ps[:, :])
                nc.tensor.matmul(
                    ctx_ps[:, :],
                    lhsT=aT[:, :],
                    rhs=fp_sb[j][:, :],
                    start=(j == 0),
                    stop=(j == M // P - 1),
                )
            ctx_sb = wpool.tile([P, D], DT, name="ctx_sb", bufs=1)
            nc.vector.tensor_scalar_mul(ctx_sb[:, :], ctx_ps[:, :], rs[:, 0:1])

            # ---- per-row pooling score s[n] = context[n, :] . wp_w ----
            tmp = wpool.tile([P, D], DT, name="tmp", bufs=1)
            nc.vector.tensor_mul(tmp[:, :], ctx_sb[:, :], wpB_sb[:, :])
            s_sb = spool.tile([P, 1], DT, name="s_sb")
            nc.vector.reduce_sum(s_sb[:, :], tmp[:, :], axis=mybir.AxisListType.X)

            # ---- outputs ----
            nc.sync.dma_start(out=ctx_out[:, :], in_=ctx_sb[:, :])
            nc.sync.dma_start(out=s_out[:, :], in_=s_sb[:, :])

    nc.finalize()
    return nc


def _prep_inputs(f_r, f_r_prime, W_w, W_b, Wp_w, Wp_b, w_w, w_b, wp_w, wp_b):
    """Host-side layout prep (transposes / broadcasts only) + sharding."""
    fpT = np.ascontiguousarray(f_r_prime.T, dtype=F32)
    fp = np.ascontiguousarray(f_r_prime, dtype=F32)
    WwT = np.ascontiguousarray(W_w.T, dtype=F32)
    WpT = np.ascontiguousarray(Wp_w.T, dtype=F32)
    Wb = np.ascontiguousarray(W_b.reshape(D, 1), dtype=F32)
    Wpb = np.ascontiguousarray(Wp_b.reshape(D, 1), dtype=F32)
    # zero-embedded w: column P (=128) holds w, so slice [P-n : 2P-n] puts w
    # in column n of the stationary operand.
    import ml_dtypes

    wext = np.zeros((D, 2 * P), dtype=F32)
    wext[:, P] = w_w.reshape(D)
    wext = wext.astype(ml_dtypes.bfloat16)
    wpB = np.broadcast_to(wp_w.reshape(1, D), (P, D)).copy()
    ident = np.eye(P, dtype=F32)

    shared = {
        "fpT": fpT,
        "fp": fp,
        "WwT": WwT,
        "WpT": WpT,
        "Wb": Wb,
        "Wpb": Wpb,
        "wext": wext,
        "wpB": wpB,
        "ident": ident,
    }
    in_maps = []
    for c in range(N_CORES):
        frT = np.ascontiguousarray(f_r[c * NP : (c + 1) * NP, :].T, dtype=F32)
        in_maps.append({"frT": frT, **shared})
    return in_maps


def _run(in_maps, **kw):
    if "nc" not in _CACHE:
        _CACHE["nc"] = build_nc()
    return run_bass_kernel_spmd(_CACHE["nc"], in_maps, list(range(N_CORES)), **kw)


def kernel(f_r, f_r_prime, W_w, W_b, Wp_w, Wp_b, w_w, w_b, wp_w, wp_b):
    in_maps = _prep_inputs(
        f_r, f_r_prime, W_w, W_b, Wp_w, Wp_b, w_w, w_b, wp_w, wp_b
    )
    res = _run(in_maps)
    ctx = np.concatenate([res.results[c]["ctx_out"] for c in range(N_CORES)], axis=0)
    s = np.concatenate(
        [res.results[c]["s_out"][:, 0] for c in range(N_CORES)], axis=0
    ).astype(np.float64)
    # final cross-shard softmax over N + pooled sum (the "all-reduce" step)
    s -= s.max()
    e = np.exp(s)
    a = (e / e.sum()).astype(F32)
    pool = a[None, :] @ ctx  # [1, D]
    return pool.astype(F32)


# revision 7
# speedup vs baseline: 1.7416x; 1.0084x over previous
"""ContextAttention (Bahdanau-style additive attention pooling) on 8 trn2 cores.

Reference math (N=M=1024, D=256):
  q = f_r @ W_w.T + W_b                     [N, D]
  k = f_r_prime @ Wp_w.T + Wp_b             [M, D]
  S[n,m]   = sum_d w_w[d] * tanh(q[n,d] + k[m,d])   (+ w_b, cancels in softmax)
  alpha    = softmax_m(S)                   [N, M]
  context  = alpha @ f_r_prime              [N, D]
  alpha_p  = softmax_n(context @ wp_w.T)    (+ wp_b, cancels)
  pool     = alpha_p.T @ context            [1, D]

Sharding: N split across 8 cores (128 rows each); f_r_prime + weights
replicated. Each core returns its context rows and per-row pooling scores;
the final softmax over N + weighted sum (the tiny cross-core reduction) is
done on host after gathering.

Device kernel layout (per core, n = 0..127 local rows):
  - d (=256) lives on partitions in 2 chunks of 128 for the tanh phase, so
    the per-row bias add q_n[d] fuses into the ScalarE ACTIVATE instruction.
  - The weighted reduction over d uses the PE with a zero-embedded stationary
    w_ext[:, 128-n : 256-n]  (w in column n, zeros elsewhere): each matmul
    writes the full [128, 512] PSUM tile but only row n receives nonzero
    values; accumulation over all n/chunks builds the [128, 1024] score tile
    exactly (zero adds are exact in fp32).
"""

import sys

sys.path.insert(0, "/opt/trn_rl_repo")

import numpy as np

import concourse.bacc as bacc
import concourse.bass as bass
import concourse.mybir as mybir
from concourse import tile
from concourse.bass_utils import run_bass_kernel_spmd

N, M, D = 1024, 1024, 256
N_CORES = 8
NP = N // N_CORES  # 128 rows per core
P = 128  # partitions
DT = mybir.dt.float32
BF = mybir.dt.bfloat16
F32 = np.float32

_CACHE = {}


def build_nc():
    nc = bacc.Bacc("TRN2", target_bir_lowering=False, debug=False, num_devices=N_CORES)

    # ---- DRAM parameters (per-core shapes) ----
    frT = nc.declare_dram_parameter("frT", [D, NP], DT, isOutput=False)
    fpT = nc.declare_dram_parameter("fpT", [D, M], DT, isOutput=False)
    fp = nc.declare_dram_parameter("fp", [M, D], DT, isOutput=False)
    WwT = nc.declare_dram_parameter("WwT", [D, D], DT, isOutput=False)
    WpT = nc.declare_dram_parameter("WpT", [D, D], DT, isOutput=False)
    Wb = nc.declare_dram_parameter("Wb", [D, 1], DT, isOutput=False)
    Wpb = nc.declare_dram_parameter("Wpb", [D, 1], DT, isOutput=False)
    wext = nc.declare_dram_parameter("wext", [D, 2 * P], BF, isOutput=False)
    wpB = nc.declare_dram_parameter("wpB", [P, D], DT, isOutput=False)
    ident = nc.declare_dram_parameter("ident", [P, P], DT, isOutput=False)

    ctx_out = nc.declare_dram_parameter("ctx_out", [NP, D], DT, isOutput=True)
    s_out = nc.declare_dram_parameter("s_out", [NP, 1], DT, isOutput=True)

    KC = D // P  # 2 contraction chunks

    with tile.TileContext(nc) as tc:
        with (
            tc.tile_pool(name="const", bufs=1) as cpool,
            tc.tile_pool(name="work", bufs=3) as wpool,
            tc.tile_pool(name="small", bufs=2) as spool,
            tc.tile_pool(name="alpha", bufs=8) as apool,
            tc.tile_pool(name="ps_qk", bufs=2, space="PSUM") as ps_qk,
            tc.tile_pool(name="ps_s", bufs=1, space="PSUM") as ps_s,
            tc.tile_pool(name="ps_tr", bufs=2, space="PSUM") as ps_tr,
            tc.tile_pool(name="ps_ctx", bufs=1, space="PSUM") as ps_ctx,
        ):
            # ---- load constants ----
            # Ordered for fastest start of phase B: kT chunk-0 deps (WpT, fpT,
            # Wpb) first, split across the sync and gpsimd DMA queues.
            # Tail-only tensors (fp, wpB, ident) load last.
            scratch = cpool.tile([1, 2], DT, name="scratch")
            nc.vector.memset(scratch[:, :], 0.0)
            # dummy 2-elem tanh: triggers the ACT table-set load during DMA
            nc.scalar.activation(
                scratch[:, :], scratch[:, :], mybir.ActivationFunctionType.Tanh
            )

            WpT_sb = [cpool.tile([P, D], DT, name=f"WpT{k}") for k in range(KC)]
            fpT_sb = [cpool.tile([P, M], DT, name=f"fpT{k}") for k in range(KC)]
            Wpb_sb = [cpool.tile([P, 1], DT, name=f"Wpb{k}") for k in range(KC)]
            frT_sb = [cpool.tile([P, NP], DT, name=f"frT{k}") for k in range(KC)]
            WwT_sb = [cpool.tile([P, D], DT, name=f"WwT{k}") for k in range(KC)]
            Wb_sb = [cpool.tile([P, 1], DT, name=f"Wb{k}") for k in range(KC)]
            wext_sb = [cpool.tile([P, 2 * P], BF, name=f"wext{k}") for k in range(KC)]
            for k in range(KC):
                nc.sync.dma_start(out=WpT_sb[k][:, :], in_=WpT[k * P : (k + 1) * P, :])
                nc.gpsimd.dma_start(
                    out=fpT_sb[k][:, :], in_=fpT[k * P : (k + 1) * P, :]
                )
                nc.sync.dma_start(out=Wpb_sb[k][:, :], in_=Wpb[k * P : (k + 1) * P, :])
            for k in range(KC):
                nc.sync.dma_start(out=frT_sb[k][:, :], in_=frT[k * P : (k + 1) * P, :])
                nc.gpsimd.dma_start(
                    out=WwT_sb[k][:, :], in_=WwT[k * P : (k + 1) * P, :]
                )
                nc.sync.dma_start(out=Wb_sb[k][:, :], in_=Wb[k * P : (k + 1) * P, :])
                nc.gpsimd.dma_start(
                    out=wext_sb[k][:, :], in_=wext[k * P : (k + 1) * P, :]
                )
            fp_sb = []
            for j in range(M // P):
                t_fp = cpool.tile([P, D], DT, name=f"fp{j}")
                (nc.sync if j % 2 else nc.gpsimd).dma_start(
                    out=t_fp[:, :], in_=fp[j * P : (j + 1) * P, :]
                )
                fp_sb.append(t_fp)
            wpB_sb = cpool.tile([P, D], DT, name="wpB")
            nc.sync.dma_start(out=wpB_sb[:, :], in_=wpB[:, :])
            ident_sb = cpool.tile([P, P], DT, name="ident")
            nc.gpsimd.dma_start(out=ident_sb[:, :], in_=ident[:, :])

            # ---- kT[d, m] = sum_d'' WpT[d'', d] * fpT[d'', m]  (+ Wpb) ----
            kT_sb = []
            for i in range(KC):
                t_kT = cpool.tile([P, M], DT, name=f"kT{i}")
                for h in range(M // 512):
                    k_ps = ps_qk.tile([P, 512], DT, name="k_ps", tag="qk")
                    for k in range(KC):
                        nc.tensor.matmul(
                            k_ps[:, :],
                            lhsT=WpT_sb[k][:, i * P : (i + 1) * P],
                            rhs=fpT_sb[k][:, h * 512 : (h + 1) * 512],
                            start=(k == 0),
                            stop=(k == KC - 1),
                        )
                    nc.vector.tensor_scalar_add(
                        t_kT[:, h * 512 : (h + 1) * 512], k_ps[:, :], Wpb_sb[i][:, 0:1]
                    )
                kT_sb.append(t_kT)

            # ---- qT[d', n] = sum_d WwT[d, d'] * frT[d, n]  (+ Wb) ----
            qT_sb = []
            for i in range(KC):
                q_ps = ps_qk.tile([P, 512], DT, name="q_ps", tag="qk")
                for k in range(KC):
                    nc.tensor.matmul(
                        q_ps[:, :NP],
                        lhsT=WwT_sb[k][:, i * P : (i + 1) * P],
                        rhs=frT_sb[k][:, :],
                        start=(k == 0),
                        stop=(k == KC - 1),
                    )
                t_qT = cpool.tile([P, NP], DT, name=f"qT{i}")
                nc.vector.tensor_scalar_add(t_qT[:, :], q_ps[:, :NP], Wb_sb[i][:, 0:1])
                qT_sb.append(t_qT)

            # ---- phase B: S[n, m] = sum_d w[d] * tanh(kT[d, m] + qT[d, n]) ----
            # (n, chunk) pairs are processed in groups of B; the per-row bias
            # add runs on DVE (tensor_scalar, 2x fp32), tanh runs on ScalarE
            # over the whole [P, B*M] group tile (amortizes the per-instr
            # overhead), and the PE reduces with bf16 inputs at full rate.
            BMAX = 8  # max (n, chunk) pairs per ACT instruction
            NITEMS = NP * KC
            # chunk-0 items first (group 0 only needs kT[0]); last groups
            # smaller to shorten the serial PE drain after the final tanh.
            items = [(n, 0) for n in range(NP)] + [(n, 1) for n in range(NP)]
            group_sizes = [8] * 31 + [4, 4]
            assert sum(group_sizes) == NITEMS
            S_ps = ps_s.tile([P, M], DT, name="S_ps")
            pos = 0
            for B in group_sizes:
                batch = items[pos : pos + B]
                first, last = pos == 0, pos + B == NITEMS
                pos += B
                Tin = wpool.tile([P, BMAX * M], DT, name="Tin", bufs=2)
                for j, (n, ci) in enumerate(batch):
                    nc.vector.tensor_scalar_add(
                        Tin[:, j * M : (j + 1) * M],
                        kT_sb[ci][:, :],
                        qT_sb[ci][:, n : n + 1],
                    )
                Tout = wpool.tile([P, BMAX * M], BF, name="Tout", bufs=2)
                nc.scalar.activation(
                    Tout[:, : B * M], Tin[:, : B * M],
                    mybir.ActivationFunctionType.Tanh,
                )
                for j, (n, ci) in enumerate(batch):
                    for h in range(M // 512):
                        nc.tensor.matmul(
                            S_ps[:, h * 512 : (h + 1) * 512],
                            lhsT=wext_sb[ci][:, P - n : 2 * P - n],
                            rhs=Tout[:, j * M + h * 512 : j * M + (h + 1) * 512],
                            start=(first and j == 0),
                            stop=(last and j == B - 1),
                        )

            # ---- softmax over m (unnormalized; row scale applied to context) ----
            mx = spool.tile([P, 1], DT, name="mx")
            nc.vector.reduce_max(mx[:, :], S_ps[:, :], axis=mybir.AxisListType.X)
            negmx = spool.tile([P, 1], DT, name="negmx")
            nc.vector.tensor_scalar_mul(negmx[:, :], mx[:, :], -1.0)
            expS = wpool.tile([P, M], DT, name="expS", bufs=1)
            sumex = spool.tile([P, 1], DT, name="sumex")
            nc.scalar.activation(
                expS[:, :],
                S_ps[:, :],
                mybir.ActivationFunctionType.Exp,
                bias=negmx[:, 0:1],
                accum_out=sumex[:, 0:1],
            )
            rs = spool.tile([P, 1], DT, name="rs")
            nc.vector.reciprocal(rs[:, :], sumex[:, :])

            # ---- context = alpha @ f_r_prime ----
            ctx_ps = ps_ctx.tile([P, D], DT, name="ctx_ps")
            for j in range(M // P):
                tr_ps = ps_tr.tile([P, P], DT, name="tr_ps")
                nc.tensor.transpose(
                    tr_ps[:, :], expS[:, j * P : (j + 1) * P], ident_sb[:, :]
                )
                aT = apool.tile([P, P], DT, name="aT")
                nc.vector.tensor_copy(aT[:, :], tr_ps[:, :])
                nc.tensor.matmul(
                    ctx_ps[:, :],
                    lhsT=aT[:, :],
                    rhs=fp_sb[j][:, :],
                    start=(j == 0),
                    stop=(j == M // P - 1),
                )
            ctx_sb = wpool.tile([P, D], DT, name="ctx_sb", bufs=1)
            nc.vector.tensor_scalar_mul(ctx_sb[:, :], ctx_ps[:, :], rs[:, 0:1])

            # ---- per-row pooling score s[n] = context[n, :] . wp_w ----
            tmp = wpool.tile([P, D], DT, name="tmp", bufs=1)
            nc.vector.tensor_mul(tmp[:, :], ctx_sb[:, :], wpB_sb[:, :])
            s_sb = spool.tile([P, 1], DT, name="s_sb")
            nc.vector.reduce_sum(s_sb[:, :], tmp[:, :], axis=mybir.AxisListType.X)

            # ---- outputs ----
            nc.sync.dma_start(out=ctx_out[:, :], in_=ctx_sb[:, :])
            nc.sync.dma_start(out=s_out[:, :], in_=s_sb[:, :])

    nc.finalize()
    return nc


def _prep_inputs(f_r, f_r_prime, W_w, W_b, Wp_w, Wp_b, w_w, w_b, wp_w, wp_b):
    """Host-side layout prep (transposes / broadcasts only) + sharding."""
    fpT = np.ascontiguousarray(f_r_prime.T, dtype=F32)
    fp = np.ascontiguousarray(f_r_prime, dtype=F32)
    WwT = np.ascontiguousarray(W_w.T, dtype=F32)
    WpT = np.ascontiguousarray(Wp_w.T, dtype=F32)
    Wb = np.ascontiguousarray(W_b.reshape(D, 1), dtype=F32)
    Wpb = np.ascontiguousarray(Wp_b.reshape(D, 1), dtype=F32)
    # zero-embedded w: column P (=128) holds w, so slice [P-n : 2P-n] puts w
    # in column n of the stationary operand.
    import ml_dtypes

    wext = np.zeros((D, 2 * P), dtype=F32)
    wext[:, P] = w_w.reshape(D)
    wext = wext.astype(ml_dtypes.bfloat16)
    wpB = np.broadcast_to(wp_w.reshape(1, D), (P, D)).copy()
    ident = np.eye(P, dtype=F32)

    shared = {
        "fpT": fpT,
        "fp": fp,
        "WwT": WwT,
        "WpT": WpT,
        "Wb": Wb,
        "Wpb": Wpb,
        "wext": wext,
        "wpB": wpB,
        "ident": ident,
    }
    in_maps = []
    for c in range(N_CORES):
        frT = np.ascontiguousarray(f_r[c * NP : (c + 1) * NP, :].T, dtype=F32)
        in_maps.append({"frT": frT, **shared})
    return in_maps


def _run(in_maps, **kw):
    if "nc" not in _CACHE:
        _CACHE["nc"] = build_nc()
    return run_bass_kernel_spmd(_CACHE["nc"], in_maps, list(range(N_CORES)), **kw)


def kernel(f_r, f_r_prime, W_w, W_b, Wp_w, Wp_b, w_w, w_b, wp_w, wp_b):
    in_maps = _prep_inputs(
        f_r, f_r_prime, W_w, W_b, Wp_w, Wp_b, w_w, w_b, wp_w, wp_b
    )
    res = _run(in_maps)
    ctx = np.concatenate([res.results[c]["ctx_out"] for c in range(N_CORES)], axis=0)
    s = np.concatenate(
        [res.results[c]["s_out"][:, 0] for c in range(N_CORES)], axis=0
    ).astype(np.float64)
    # final cross-shard softmax over N + pooled sum (the "all-reduce" step)
    s -= s.max()
    e = np.exp(s)
    a = (e / e.sum()).astype(F32)
    pool = a[None, :] @ ctx  # [1, D]
    return pool.astype(F32)


# revision 8
# speedup vs baseline: 1.7569x; 1.0088x over previous
"""ContextAttention (Bahdanau-style additive attention pooling) on 8 trn2 cores.

Reference math (N=M=1024, D=256):
  q = f_r @ W_w.T + W_b                     [N, D]
  k = f_r_prime @ Wp_w.T + Wp_b             [M, D]
  S[n,m]   = sum_d w_w[d] * tanh(q[n,d] + k[m,d])   (+ w_b, cancels in softmax)
  alpha    = softmax_m(S)                   [N, M]
  context  = alpha @ f_r_prime              [N, D]
  alpha_p  = softmax_n(context @ wp_w.T)    (+ wp_b, cancels)
  pool     = alpha_p.T @ context            [1, D]

Sharding: N split across 8 cores (128 rows each); f_r_prime + weights
replicated. Each core returns its context rows and per-row pooling scores;
the final softmax over N + weighted sum (the tiny cross-core reduction) is
done on host after gathering.

Device kernel layout (per core, n = 0..127 local rows):
  - d (=256) lives on partitions in 2 chunks of 128 for the tanh phase, so
    the per-row bias add q_n[d] fuses into the ScalarE ACTIVATE instruction.
  - The weighted reduction over d uses the PE with a zero-embedded stationary
    w_ext[:, 128-n : 256-n]  (w in column n, zeros elsewhere): each matmul
    writes the full [128, 512] PSUM tile but only row n receives nonzero
    values; accumulation over all n/chunks builds the [128, 1024] score tile
    exactly (zero adds are exact in fp32).
"""

import sys

sys.path.insert(0, "/opt/trn_rl_repo")

import numpy as np

import concourse.bacc as bacc
import concourse.bass as bass
import concourse.mybir as mybir
from concourse import tile
from concourse.bass_utils import run_bass_kernel_spmd

N, M, D = 1024, 1024, 256
N_CORES = 8
NP = N // N_CORES  # 128 rows per core
P = 128  # partitions
DT = mybir.dt.float32
BF = mybir.dt.bfloat16
F32 = np.float32

_CACHE = {}


def build_nc():
    nc = bacc.Bacc("TRN2", target_bir_lowering=False, debug=False, num_devices=N_CORES)

    # ---- DRAM parameters (per-core shapes) ----
    frT = nc.declare_dram_parameter("frT", [D, NP], DT, isOutput=False)
    fpT = nc.declare_dram_parameter("fpT", [D, M], DT, isOutput=False)
    fp = nc.declare_dram_parameter("fp", [M, D], DT, isOutput=False)
    WwT = nc.declare_dram_parameter("WwT", [D, D], DT, isOutput=False)
    WpT = nc.declare_dram_parameter("WpT", [D, D], DT, isOutput=False)
    Wb = nc.declare_dram_parameter("Wb", [D, 1], DT, isOutput=False)
    Wpb = nc.declare_dram_parameter("Wpb", [D, 1], DT, isOutput=False)
    wext = nc.declare_dram_parameter("wext", [D, 2 * P], BF, isOutput=False)
    wpB = nc.declare_dram_parameter("wpB", [P, D], DT, isOutput=False)
    ident = nc.declare_dram_parameter("ident", [P, P], DT, isOutput=False)

    ctx_out = nc.declare_dram_parameter("ctx_out", [NP, D], DT, isOutput=True)
    s_out = nc.declare_dram_parameter("s_out", [NP, 1], DT, isOutput=True)

    KC = D // P  # 2 contraction chunks

    with tile.TileContext(nc) as tc:
        with (
            tc.tile_pool(name="const", bufs=1) as cpool,
            tc.tile_pool(name="work", bufs=3) as wpool,
            tc.tile_pool(name="small", bufs=2) as spool,
            tc.tile_pool(name="alpha", bufs=8) as apool,
            tc.tile_pool(name="ps_qk", bufs=2, space="PSUM") as ps_qk,
            tc.tile_pool(name="ps_s", bufs=1, space="PSUM") as ps_s,
            tc.tile_pool(name="ps_tr", bufs=2, space="PSUM") as ps_tr,
            tc.tile_pool(name="ps_ctx", bufs=1, space="PSUM") as ps_ctx,
        ):
            # ---- load constants ----
            # Ordered for fastest start of phase B: kT chunk-0 deps (WpT, fpT,
            # Wpb) first, split across the sync and gpsimd DMA queues.
            # Tail-only tensors (fp, wpB, ident) load last.
            scratch = cpool.tile([1, 2], DT, name="scratch")
            nc.vector.memset(scratch[:, :], 0.0)
            # dummy 2-elem tanh: triggers the ACT table-set load during DMA
            nc.scalar.activation(
                scratch[:, :], scratch[:, :], mybir.ActivationFunctionType.Tanh
            )

            WpT_sb = [cpool.tile([P, D], DT, name=f"WpT{k}") for k in range(KC)]
            fpT_sb = [cpool.tile([P, M], DT, name=f"fpT{k}") for k in range(KC)]
            Wpb_sb = [cpool.tile([P, 1], DT, name=f"Wpb{k}") for k in range(KC)]
            frT_sb = [cpool.tile([P, NP], DT, name=f"frT{k}") for k in range(KC)]
            WwT_sb = [cpool.tile([P, D], DT, name=f"WwT{k}") for k in range(KC)]
            Wb_sb = [cpool.tile([P, 1], DT, name=f"Wb{k}") for k in range(KC)]
            wext_sb = [cpool.tile([P, 2 * P], BF, name=f"wext{k}") for k in range(KC)]
            for k in range(KC):
                nc.sync.dma_start(out=WpT_sb[k][:, :], in_=WpT[k * P : (k + 1) * P, :])
                nc.gpsimd.dma_start(
                    out=fpT_sb[k][:, :], in_=fpT[k * P : (k + 1) * P, :]
                )
                nc.sync.dma_start(out=Wpb_sb[k][:, :], in_=Wpb[k * P : (k + 1) * P, :])
            for k in range(KC):
                nc.sync.dma_start(out=frT_sb[k][:, :], in_=frT[k * P : (k + 1) * P, :])
                nc.gpsimd.dma_start(
                    out=WwT_sb[k][:, :], in_=WwT[k * P : (k + 1) * P, :]
                )
                nc.sync.dma_start(out=Wb_sb[k][:, :], in_=Wb[k * P : (k + 1) * P, :])
                nc.gpsimd.dma_start(
                    out=wext_sb[k][:, :], in_=wext[k * P : (k + 1) * P, :]
                )
            fp_sb = []
            for j in range(M // P):
                t_fp = cpool.tile([P, D], DT, name=f"fp{j}")
                (nc.sync if j % 2 else nc.gpsimd).dma_start(
                    out=t_fp[:, :], in_=fp[j * P : (j + 1) * P, :]
                )
                fp_sb.append(t_fp)
            wpB_sb = cpool.tile([P, D], DT, name="wpB")
            nc.sync.dma_start(out=wpB_sb[:, :], in_=wpB[:, :])
            ident_sb = cpool.tile([P, P], DT, name="ident")
            nc.gpsimd.dma_start(out=ident_sb[:, :], in_=ident[:, :])

            # ---- kT[d, m] = Wp_w @ f_r_prime^T (+Wpb); qT[d, n] likewise.
            # Chunk-0 of both is computed first so phase B group 0 unblocks
            # as early as possible.
            kT_sb = [cpool.tile([P, M], DT, name=f"kT{i}") for i in range(KC)]
            qT_sb = [cpool.tile([P, NP], DT, name=f"qT{i}") for i in range(KC)]
            for i in range(KC):
                for h in range(M // 512):
                    k_ps = ps_qk.tile([P, 512], DT, name="k_ps", tag="qk")
                    for k in range(KC):
                        nc.tensor.matmul(
                            k_ps[:, :],
                            lhsT=WpT_sb[k][:, i * P : (i + 1) * P],
                            rhs=fpT_sb[k][:, h * 512 : (h + 1) * 512],
                            start=(k == 0),
                            stop=(k == KC - 1),
                        )
                    nc.vector.tensor_scalar_add(
                        kT_sb[i][:, h * 512 : (h + 1) * 512],
                        k_ps[:, :],
                        Wpb_sb[i][:, 0:1],
                    )
                q_ps = ps_qk.tile([P, 512], DT, name="q_ps", tag="qk")
                for k in range(KC):
                    nc.tensor.matmul(
                        q_ps[:, :NP],
                        lhsT=WwT_sb[k][:, i * P : (i + 1) * P],
                        rhs=frT_sb[k][:, :],
                        start=(k == 0),
                        stop=(k == KC - 1),
                    )
                nc.vector.tensor_scalar_add(
                    qT_sb[i][:, :], q_ps[:, :NP], Wb_sb[i][:, 0:1]
                )

            # ---- phase B: S[n, m] = sum_d w[d] * tanh(kT[d, m] + qT[d, n]) ----
            # (n, chunk) pairs are processed in groups of B; the per-row bias
            # add runs on DVE (tensor_scalar, 2x fp32), tanh runs on ScalarE
            # over the whole [P, B*M] group tile (amortizes the per-instr
            # overhead), and the PE reduces with bf16 inputs at full rate.
            BMAX = 8  # max (n, chunk) pairs per ACT instruction
            NITEMS = NP * KC
            # chunk-0 items first (group 0 only needs kT[0]); last groups
            # smaller to shorten the serial PE drain after the final tanh.
            items = [(n, 0) for n in range(NP)] + [(n, 1) for n in range(NP)]
            group_sizes = [8] * 31 + [4, 2, 2]
            assert sum(group_sizes) == NITEMS
            S_ps = ps_s.tile([P, M], DT, name="S_ps")
            pos = 0
            for B in group_sizes:
                batch = items[pos : pos + B]
                first, last = pos == 0, pos + B == NITEMS
                pos += B
                Tin = wpool.tile([P, BMAX * M], BF, name="Tin", bufs=2)
                for j, (n, ci) in enumerate(batch):
                    nc.vector.tensor_scalar_add(
                        Tin[:, j * M : (j + 1) * M],
                        kT_sb[ci][:, :],
                        qT_sb[ci][:, n : n + 1],
                    )
                Tout = wpool.tile([P, BMAX * M], BF, name="Tout", bufs=2)
                nc.scalar.activation(
                    Tout[:, : B * M], Tin[:, : B * M],
                    mybir.ActivationFunctionType.Tanh,
                )
                for j, (n, ci) in enumerate(batch):
                    for h in range(M // 512):
                        nc.tensor.matmul(
                            S_ps[:, h * 512 : (h + 1) * 512],
                            lhsT=wext_sb[ci][:, P - n : 2 * P - n],
                            rhs=Tout[:, j * M + h * 512 : j * M + (h + 1) * 512],
                            start=(first and j == 0),
                            stop=(last and j == B - 1),
                        )

            # ---- softmax over m (unnormalized; row scale applied to context) ----
            # |S| <= sum|w| ~ 8, so exp(S) is fp32-safe without the usual
            # max-subtraction (softmax is shift-invariant); two half-width
            # instructions let the first transposes start earlier.
            expS = wpool.tile([P, M], DT, name="expS", bufs=1)
            sumex = spool.tile([P, 2], DT, name="sumex")
            for h in range(2):
                nc.scalar.activation(
                    expS[:, h * 512 : (h + 1) * 512],
                    S_ps[:, h * 512 : (h + 1) * 512],
                    mybir.ActivationFunctionType.Exp,
                    accum_out=sumex[:, h : h + 1],
                )
            sumt = spool.tile([P, 1], DT, name="sumt")
            nc.vector.tensor_add(sumt[:, :], sumex[:, 0:1], sumex[:, 1:2])
            rs = spool.tile([P, 1], DT, name="rs")
            nc.vector.reciprocal(rs[:, :], sumt[:, :])

            # ---- context = alpha @ f_r_prime ----
            ctx_ps = ps_ctx.tile([P, D], DT, name="ctx_ps")
            for j in range(M // P):
                tr_ps = ps_tr.tile([P, P], DT, name="tr_ps")
                nc.tensor.transpose(
                    tr_ps[:, :], expS[:, j * P : (j + 1) * P], ident_sb[:, :]
                )
                aT = apool.tile([P, P], DT, name="aT")
                nc.vector.tensor_copy(aT[:, :], tr_ps[:, :])
                nc.tensor.matmul(
                    ctx_ps[:, :],
                    lhsT=aT[:, :],
                    rhs=fp_sb[j][:, :],
                    start=(j == 0),
                    stop=(j == M // P - 1),
                )
            ctx_sb = wpool.tile([P, D], DT, name="ctx_sb", bufs=1)
            nc.vector.tensor_scalar_mul(ctx_sb[:, :], ctx_ps[:, :], rs[:, 0:1])

            # ---- per-row pooling score s[n] = context[n, :] . wp_w ----
            tmp = wpool.tile([P, D], DT, name="tmp", bufs=1)
            nc.vector.tensor_mul(tmp[:, :], ctx_sb[:, :], wpB_sb[:, :])
            s_sb = spool.tile([P, 1], DT, name="s_sb")
            nc.vector.reduce_sum(s_sb[:, :], tmp[:, :], axis=mybir.AxisListType.X)

            # ---- outputs ----
            nc.sync.dma_start(out=ctx_out[:, :], in_=ctx_sb[:, :])
            nc.sync.dma_start(out=s_out[:, :], in_=s_sb[:, :])

    nc.finalize()
    return nc


def _prep_inputs(f_r, f_r_prime, W_w, W_b, Wp_w, Wp_b, w_w, w_b, wp_w, wp_b):
    """Host-side layout prep (transposes / broadcasts only) + sharding."""
    fpT = np.ascontiguousarray(f_r_prime.T, dtype=F32)
    fp = np.ascontiguousarray(f_r_prime, dtype=F32)
    WwT = np.ascontiguousarray(W_w.T, dtype=F32)
    WpT = np.ascontiguousarray(Wp_w.T, dtype=F32)
    Wb = np.ascontiguousarray(W_b.reshape(D, 1), dtype=F32)
    Wpb = np.ascontiguousarray(Wp_b.reshape(D, 1), dtype=F32)
    # zero-embedded w: column P (=128) holds w, so slice [P-n : 2P-n] puts w
    # in column n of the stationary operand.
    import ml_dtypes

    wext = np.zeros((D, 2 * P), dtype=F32)
    wext[:, P] = w_w.reshape(D)
    wext = wext.astype(ml_dtypes.bfloat16)
    wpB = np.broadcast_to(wp_w.reshape(1, D), (P, D)).copy()
    ident = np.eye(P, dtype=F32)

    shared = {
        "fpT": fpT,
        "fp": fp,
        "WwT": WwT,
        "WpT": WpT,
        "Wb": Wb,
        "Wpb": Wpb,
        "wext": wext,
        "wpB": wpB,
        "ident": ident,
    }
    in_maps = []
    for c in range(N_CORES):
        frT = np.ascontiguousarray(f_r[c * NP : (c + 1) * NP, :].T, dtype=F32)
        in_maps.append({"frT": frT, **shared})
    return in_maps


def _run(in_maps, **kw):
    if "nc" not in _CACHE:
        _CACHE["nc"] = build_nc()
    return run_bass_kernel_spmd(_CACHE["nc"], in_maps, list(range(N_CORES)), **kw)


def kernel(f_r, f_r_prime, W_w, W_b, Wp_w, Wp_b, w_w, w_b, wp_w, wp_b):
    in_maps = _prep_inputs(
        f_r, f_r_prime, W_w, W_b, Wp_w, Wp_b, w_w, w_b, wp_w, wp_b
    )
    res = _run(in_maps)
    ctx = np.concatenate([res.results[c]["ctx_out"] for c in range(N_CORES)], axis=0)
    s = np.concatenate(
        [res.results[c]["s_out"][:, 0] for c in range(N_CORES)], axis=0
    ).astype(np.float64)
    # final cross-shard softmax over N + pooled sum (the "all-reduce" step)
    s -= s.max()
    e = np.exp(s)
    a = (e / e.sum()).astype(F32)
    pool = a[None, :] @ ctx  # [1, D]
    return pool.astype(F32)


# revision 9
# speedup vs baseline: 1.7816x; 1.0140x over previous
"""ContextAttention (Bahdanau-style additive attention pooling) on 8 trn2 cores.

Reference math (N=M=1024, D=256):
  q = f_r @ W_w.T + W_b                     [N, D]
  k = f_r_prime @ Wp_w.T + Wp_b             [M, D]
  S[n,m]   = sum_d w_w[d] * tanh(q[n,d] + k[m,d])   (+ w_b, cancels in softmax)
  alpha    = softmax_m(S)                   [N, M]
  context  = alpha @ f_r_prime              [N, D]
  alpha_p  = softmax_n(context @ wp_w.T)    (+ wp_b, cancels)
  pool     = alpha_p.T @ context            [1, D]

Sharding: N split across 8 cores (128 rows each); f_r_prime + weights
replicated. Each core returns its context rows and per-row pooling scores;
the final softmax over N + weighted sum (the tiny cross-core reduction) is
done on host after gathering.

Device kernel layout (per core, n = 0..127 local rows):
  - d (=256) lives on partitions in 2 chunks of 128 for the tanh phase, so
    the per-row bias add q_n[d] fuses into the ScalarE ACTIVATE instruction.
  - The weighted reduction over d uses the PE with a zero-embedded stationary
    w_ext[:, 128-n : 256-n]  (w in column n, zeros elsewhere): each matmul
    writes the full [128, 512] PSUM tile but only row n receives nonzero
    values; accumulation over all n/chunks builds the [128, 1024] score tile
    exactly (zero adds are exact in fp32).
"""

import sys

sys.path.insert(0, "/opt/trn_rl_repo")

import numpy as np

import concourse.bacc as bacc
import concourse.bass as bass
import concourse.mybir as mybir
from concourse import tile
from concourse.bass_utils import run_bass_kernel_spmd

N, M, D = 1024, 1024, 256
N_CORES = 8
NP = N // N_CORES  # 128 rows per core
P = 128  # partitions
DT = mybir.dt.float32
BF = mybir.dt.bfloat16
F32 = np.float32

_CACHE = {}


def build_nc():
    nc = bacc.Bacc("TRN2", target_bir_lowering=False, debug=False, num_devices=N_CORES)

    # ---- DRAM parameters (per-core shapes) ----
    frT = nc.declare_dram_parameter("frT", [D, NP], DT, isOutput=False)
    fpT = nc.declare_dram_parameter("fpT", [D, M], DT, isOutput=False)
    fp = nc.declare_dram_parameter("fp", [M, D], DT, isOutput=False)
    WwT = nc.declare_dram_parameter("WwT", [D, D], DT, isOutput=False)
    WpT = nc.declare_dram_parameter("WpT", [D, D], DT, isOutput=False)
    Wb = nc.declare_dram_parameter("Wb", [D, 1], DT, isOutput=False)
    Wpb = nc.declare_dram_parameter("Wpb", [D, 1], DT, isOutput=False)
    wext = nc.declare_dram_parameter("wext", [D, 2 * P], BF, isOutput=False)
    wpB = nc.declare_dram_parameter("wpB", [P, D], DT, isOutput=False)
    ident = nc.declare_dram_parameter("ident", [P, P], DT, isOutput=False)

    ctx_out = nc.declare_dram_parameter("ctx_out", [NP, D], DT, isOutput=True)
    s_out = nc.declare_dram_parameter("s_out", [NP, 1], DT, isOutput=True)

    KC = D // P  # 2 contraction chunks

    with tile.TileContext(nc) as tc:
        with (
            tc.tile_pool(name="const", bufs=1) as cpool,
            tc.tile_pool(name="work", bufs=3) as wpool,
            tc.tile_pool(name="small", bufs=2) as spool,
            tc.tile_pool(name="alpha", bufs=8) as apool,
            tc.tile_pool(name="ps_qk", bufs=2, space="PSUM") as ps_qk,
            tc.tile_pool(name="ps_s", bufs=1, space="PSUM") as ps_s,
            tc.tile_pool(name="ps_tr", bufs=2, space="PSUM") as ps_tr,
            tc.tile_pool(name="ps_ctx", bufs=1, space="PSUM") as ps_ctx,
        ):
            # ---- load constants ----
            # Ordered for fastest start of phase B: kT chunk-0 deps (WpT, fpT,
            # Wpb) first, split across the sync and gpsimd DMA queues.
            # Tail-only tensors (fp, wpB, ident) load last.
            scratch = cpool.tile([1, 2], DT, name="scratch")
            nc.vector.memset(scratch[:, :], 0.0)
            # dummy 2-elem tanh: triggers the ACT table-set load during DMA
            nc.scalar.activation(
                scratch[:, :], scratch[:, :], mybir.ActivationFunctionType.Tanh
            )

            WpT_sb = [cpool.tile([P, D], DT, name=f"WpT{k}") for k in range(KC)]
            fpT_sb = [cpool.tile([P, M], DT, name=f"fpT{k}") for k in range(KC)]
            Wpb_sb = [cpool.tile([P, 1], DT, name=f"Wpb{k}") for k in range(KC)]
            frT_sb = [cpool.tile([P, NP], DT, name=f"frT{k}") for k in range(KC)]
            WwT_sb = [cpool.tile([P, D], DT, name=f"WwT{k}") for k in range(KC)]
            Wb_sb = [cpool.tile([P, 1], DT, name=f"Wb{k}") for k in range(KC)]
            wext_sb = [cpool.tile([P, 2 * P], BF, name=f"wext{k}") for k in range(KC)]
            for k in range(KC):
                nc.sync.dma_start(out=frT_sb[k][:, :], in_=frT[k * P : (k + 1) * P, :])
                nc.gpsimd.dma_start(
                    out=WwT_sb[k][:, :], in_=WwT[k * P : (k + 1) * P, :]
                )
            nc.sync.dma_start(out=Wb_sb[0][:, :], in_=Wb[0:P, :])
            nc.gpsimd.dma_start(out=Wb_sb[1][:, :], in_=Wb[P : 2 * P, :])
            for k in range(KC):
                nc.sync.dma_start(out=WpT_sb[k][:, :], in_=WpT[k * P : (k + 1) * P, :])
                nc.gpsimd.dma_start(
                    out=fpT_sb[k][:, :], in_=fpT[k * P : (k + 1) * P, :]
                )
                nc.sync.dma_start(out=Wpb_sb[k][:, :], in_=Wpb[k * P : (k + 1) * P, :])
                nc.gpsimd.dma_start(
                    out=wext_sb[k][:, :], in_=wext[k * P : (k + 1) * P, :]
                )
            fp_sb = []
            for j in range(M // P):
                t_fp = cpool.tile([P, D], DT, name=f"fp{j}")
                (nc.sync if j % 2 else nc.gpsimd).dma_start(
                    out=t_fp[:, :], in_=fp[j * P : (j + 1) * P, :]
                )
                fp_sb.append(t_fp)
            wpB_sb = cpool.tile([P, D], DT, name="wpB")
            nc.sync.dma_start(out=wpB_sb[:, :], in_=wpB[:, :])
            ident_sb = cpool.tile([P, P], DT, name="ident")
            nc.gpsimd.dma_start(out=ident_sb[:, :], in_=ident[:, :])

            # ---- kT[d, m] = Wp_w @ f_r_prime^T (+Wpb); qT[d, n] likewise.
            # Chunk-0 of both is computed first so phase B group 0 unblocks
            # as early as possible.
            kT_sb = [cpool.tile([P, M], DT, name=f"kT{i}") for i in range(KC)]
            qT_sb = [cpool.tile([P, NP], DT, name=f"qT{i}") for i in range(KC)]
            for i in range(KC):
                q_ps = ps_qk.tile([P, 512], DT, name="q_ps", tag="qk")
                for k in range(KC):
                    nc.tensor.matmul(
                        q_ps[:, :NP],
                        lhsT=WwT_sb[k][:, i * P : (i + 1) * P],
                        rhs=frT_sb[k][:, :],
                        start=(k == 0),
                        stop=(k == KC - 1),
                    )
                nc.vector.tensor_scalar_add(
                    qT_sb[i][:, :], q_ps[:, :NP], Wb_sb[i][:, 0:1]
                )
                for h in range(M // 512):
                    k_ps = ps_qk.tile([P, 512], DT, name="k_ps", tag="qk")
                    for k in range(KC):
                        nc.tensor.matmul(
                            k_ps[:, :],
                            lhsT=WpT_sb[k][:, i * P : (i + 1) * P],
                            rhs=fpT_sb[k][:, h * 512 : (h + 1) * 512],
                            start=(k == 0),
                            stop=(k == KC - 1),
                        )
                    nc.vector.tensor_scalar_add(
                        kT_sb[i][:, h * 512 : (h + 1) * 512],
                        k_ps[:, :],
                        Wpb_sb[i][:, 0:1],
                    )

            # ---- phase B: S[n, m] = sum_d w[d] * tanh(kT[d, m] + qT[d, n]) ----
            # (n, chunk) pairs are processed in groups of B; the per-row bias
            # add runs on DVE (tensor_scalar, 2x fp32), tanh runs on ScalarE
            # over the whole [P, B*M] group tile (amortizes the per-instr
            # overhead), and the PE reduces with bf16 inputs at full rate.
            BMAX = 8  # max (n, chunk) pairs per ACT instruction
            NITEMS = NP * KC
            # chunk-0 items first (group 0 only needs kT[0]); last groups
            # smaller to shorten the serial PE drain after the final tanh.
            items = [(n, 0) for n in range(NP)] + [(n, 1) for n in range(NP)]
            group_sizes = [2, 2, 4, 4] + [8] * 29 + [4, 4, 2, 2]
            assert sum(group_sizes) == NITEMS
            S_ps = ps_s.tile([P, M], DT, name="S_ps")
            pos = 0
            for B in group_sizes:
                batch = items[pos : pos + B]
                first, last = pos == 0, pos + B == NITEMS
                pos += B
                Tin = wpool.tile([P, BMAX * M], BF, name="Tin", bufs=2)
                for j, (n, ci) in enumerate(batch):
                    nc.vector.tensor_scalar_add(
                        Tin[:, j * M : (j + 1) * M],
                        kT_sb[ci][:, :],
                        qT_sb[ci][:, n : n + 1],
                    )
                Tout = wpool.tile([P, BMAX * M], BF, name="Tout", bufs=2)
                nc.scalar.activation(
                    Tout[:, : B * M], Tin[:, : B * M],
                    mybir.ActivationFunctionType.Tanh,
                )
                for j, (n, ci) in enumerate(batch):
                    for h in range(M // 512):
                        nc.tensor.matmul(
                            S_ps[:, h * 512 : (h + 1) * 512],
                            lhsT=wext_sb[ci][:, P - n : 2 * P - n],
                            rhs=Tout[:, j * M + h * 512 : j * M + (h + 1) * 512],
                            start=(first and j == 0),
                            stop=(last and j == B - 1),
                        )

            # ---- softmax over m (unnormalized; row scale applied to context) ----
            # |S| <= sum|w| ~ 8, so exp(S) is fp32-safe without the usual
            # max-subtraction (softmax is shift-invariant); two half-width
            # instructions let the first transposes start earlier.
            expS = wpool.tile([P, M], DT, name="expS", bufs=1)
            sumex = spool.tile([P, 2], DT, name="sumex")
            for h in range(2):
                nc.scalar.activation(
                    expS[:, h * 512 : (h + 1) * 512],
                    S_ps[:, h * 512 : (h + 1) * 512],
                    mybir.ActivationFunctionType.Exp,
                    accum_out=sumex[:, h : h + 1],
                )
            sumt = spool.tile([P, 1], DT, name="sumt")
            nc.vector.tensor_add(sumt[:, :], sumex[:, 0:1], sumex[:, 1:2])
            rs = spool.tile([P, 1], DT, name="rs")
            nc.vector.reciprocal(rs[:, :], sumt[:, :])

            # ---- context = alpha @ f_r_prime ----
            ctx_ps = ps_ctx.tile([P, D], DT, name="ctx_ps")
            for j in range(M // P):
                tr_ps = ps_tr.tile([P, P], DT, name="tr_ps")
                nc.tensor.transpose(
                    tr_ps[:, :], expS[:, j * P : (j + 1) * P], ident_sb[:, :]
                )
                aT = apool.tile([P, P], DT, name="aT")
                nc.vector.tensor_copy(aT[:, :], tr_ps[:, :])
                nc.tensor.matmul(
                    ctx_ps[:, :],
                    lhsT=aT[:, :],
                    rhs=fp_sb[j][:, :],
                    start=(j == 0),
                    stop=(j == M // P - 1),
                )
            ctx_sb = wpool.tile([P, D], DT, name="ctx_sb", bufs=1)
            nc.vector.tensor_scalar_mul(ctx_sb[:, :], ctx_ps[:, :], rs[:, 0:1])

            # ---- per-row pooling score s[n] = context[n, :] . wp_w ----
            tmp = wpool.tile([P, D], DT, name="tmp", bufs=1)
            nc.vector.tensor_mul(tmp[:, :], ctx_sb[:, :], wpB_sb[:, :])
            s_sb = spool.tile([P, 1], DT, name="s_sb")
            nc.vector.reduce_sum(s_sb[:, :], tmp[:, :], axis=mybir.AxisListType.X)

            # ---- outputs ----
            nc.sync.dma_start(out=ctx_out[:, :], in_=ctx_sb[:, :])
            nc.sync.dma_start(out=s_out[:, :], in_=s_sb[:, :])

    nc.finalize()
    return nc


def _prep_inputs(f_r, f_r_prime, W_w, W_b, Wp_w, Wp_b, w_w, w_b, wp_w, wp_b):
    """Host-side layout prep (transposes / broadcasts only) + sharding."""
    fpT = np.ascontiguousarray(f_r_prime.T, dtype=F32)
    fp = np.ascontiguousarray(f_r_prime, dtype=F32)
    WwT = np.ascontiguousarray(W_w.T, dtype=F32)
    WpT = np.ascontiguousarray(Wp_w.T, dtype=F32)
    Wb = np.ascontiguousarray(W_b.reshape(D, 1), dtype=F32)
    Wpb = np.ascontiguousarray(Wp_b.reshape(D, 1), dtype=F32)
    # zero-embedded w: column P (=128) holds w, so slice [P-n : 2P-n] puts w
    # in column n of the stationary operand.
    import ml_dtypes

    wext = np.zeros((D, 2 * P), dtype=F32)
    wext[:, P] = w_w.reshape(D)
    wext = wext.astype(ml_dtypes.bfloat16)
    wpB = np.broadcast_to(wp_w.reshape(1, D), (P, D)).copy()
    ident = np.eye(P, dtype=F32)

    shared = {
        "fpT": fpT,
        "fp": fp,
        "WwT": WwT,
        "WpT": WpT,
        "Wb": Wb,
        "Wpb": Wpb,
        "wext": wext,
        "wpB": wpB,
        "ident": ident,
    }
    in_maps = []
    for c in range(N_CORES):
        frT = np.ascontiguousarray(f_r[c * NP : (c + 1) * NP, :].T, dtype=F32)
        in_maps.append({"frT": frT, **shared})
    return in_maps


def _run(in_maps, **kw):
    if "nc" not in _CACHE:
        _CACHE["nc"] = build_nc()
    return run_bass_kernel_spmd(_CACHE["nc"], in_maps, list(range(N_CORES)), **kw)


def kernel(f_r, f_r_prime, W_w, W_b, Wp_w, Wp_b, w_w, w_b, wp_w, wp_b):
    in_maps = _prep_inputs(
        f_r, f_r_prime, W_w, W_b, Wp_w, Wp_b, w_w, w_b, wp_w, wp_b
    )
    res = _run(in_maps)
    ctx = np.concatenate([res.results[c]["ctx_out"] for c in range(N_CORES)], axis=0)
    s = np.concatenate(
        [res.results[c]["s_out"][:, 0] for c in range(N_CORES)], axis=0
    ).astype(np.float64)
    # final cross-shard softmax over N + pooled sum (the "all-reduce" step)
    s -= s.max()
    e = np.exp(s)
    a = (e / e.sum()).astype(F32)
    pool = a[None, :] @ ctx  # [1, D]
    return pool.astype(F32)


# revision 10
# speedup vs baseline: 1.8098x; 1.0159x over previous
"""ContextAttention (Bahdanau-style additive attention pooling) on 8 trn2 cores.

Reference math (N=M=1024, D=256):
  q = f_r @ W_w.T + W_b                     [N, D]
  k = f_r_prime @ Wp_w.T + Wp_b             [M, D]
  S[n,m]   = sum_d w_w[d] * tanh(q[n,d] + k[m,d])   (+ w_b, cancels in softmax)
  alpha    = softmax_m(S)                   [N, M]
  context  = alpha @ f_r_prime              [N, D]
  alpha_p  = softmax_n(context @ wp_w.T)    (+ wp_b, cancels)
  pool     = alpha_p.T @ context            [1, D]

Sharding: N split across 8 cores (128 rows each); f_r_prime + weights
replicated. Each core returns its context rows and per-row pooling scores;
the final softmax over N + weighted sum (the tiny cross-core reduction) is
done on host after gathering.

Device kernel layout (per core, n = 0..127 local rows):
  - d (=256) lives on partitions in 2 chunks of 128 for the tanh phase, so
    the per-row bias add q_n[d] fuses into the ScalarE ACTIVATE instruction.
  - The weighted reduction over d uses the PE with a zero-embedded stationary
    w_ext[:, 128-n : 256-n]  (w in column n, zeros elsewhere): each matmul
    writes the full [128, 512] PSUM tile but only row n receives nonzero
    values; accumulation over all n/chunks builds the [128, 1024] score tile
    exactly (zero adds are exact in fp32).
"""

import sys

sys.path.insert(0, "/opt/trn_rl_repo")

import numpy as np

import concourse.bacc as bacc
import concourse.bass as bass
import concourse.mybir as mybir
from concourse import tile
from concourse.bass_utils import run_bass_kernel_spmd

N, M, D = 1024, 1024, 256
N_CORES = 8
NP = N // N_CORES  # 128 rows per core
P = 128  # partitions
DT = mybir.dt.float32
BF = mybir.dt.bfloat16
F32 = np.float32

_CACHE = {}


def build_nc():
    nc = bacc.Bacc("TRN2", target_bir_lowering=False, debug=False, num_devices=N_CORES)

    # ---- DRAM parameters (per-core shapes) ----
    frT = nc.declare_dram_parameter("frT", [D, NP], BF, isOutput=False)
    fpT = nc.declare_dram_parameter("fpT", [D, M], BF, isOutput=False)
    fp = nc.declare_dram_parameter("fp", [M, D], DT, isOutput=False)
    WwT = nc.declare_dram_parameter("WwT", [D, D], BF, isOutput=False)
    WpT = nc.declare_dram_parameter("WpT", [D, D], BF, isOutput=False)
    Wb = nc.declare_dram_parameter("Wb", [D, 1], DT, isOutput=False)
    Wpb = nc.declare_dram_parameter("Wpb", [D, 1], DT, isOutput=False)
    wext = nc.declare_dram_parameter("wext", [D, 2 * P], BF, isOutput=False)
    wpB = nc.declare_dram_parameter("wpB", [P, D], DT, isOutput=False)
    ident = nc.declare_dram_parameter("ident", [P, P], DT, isOutput=False)

    ctx_out = nc.declare_dram_parameter("ctx_out", [NP, D], DT, isOutput=True)
    s_out = nc.declare_dram_parameter("s_out", [NP, 1], DT, isOutput=True)

    KC = D // P  # 2 contraction chunks

    with tile.TileContext(nc) as tc:
        with (
            tc.tile_pool(name="const", bufs=1) as cpool,
            tc.tile_pool(name="work", bufs=3) as wpool,
            tc.tile_pool(name="small", bufs=2) as spool,
            tc.tile_pool(name="alpha", bufs=8) as apool,
            tc.tile_pool(name="ps_qk", bufs=2, space="PSUM") as ps_qk,
            tc.tile_pool(name="ps_s", bufs=1, space="PSUM") as ps_s,
            tc.tile_pool(name="ps_tr", bufs=2, space="PSUM") as ps_tr,
            tc.tile_pool(name="ps_ctx", bufs=1, space="PSUM") as ps_ctx,
        ):
            # ---- load constants ----
            # Ordered for fastest start of phase B: kT chunk-0 deps (WpT, fpT,
            # Wpb) first, split across the sync and gpsimd DMA queues.
            # Tail-only tensors (fp, wpB, ident) load last.
            scratch = cpool.tile([1, 2], DT, name="scratch")
            nc.vector.memset(scratch[:, :], 0.0)
            # dummy 2-elem tanh: triggers the ACT table-set load during DMA
            nc.scalar.activation(
                scratch[:, :], scratch[:, :], mybir.ActivationFunctionType.Tanh
            )

            WpT_sb = [cpool.tile([P, D], BF, name=f"WpT{k}") for k in range(KC)]
            fpT_sb = [cpool.tile([P, M], BF, name=f"fpT{k}") for k in range(KC)]
            Wpb_sb = [cpool.tile([P, 1], DT, name=f"Wpb{k}") for k in range(KC)]
            frT_sb = [cpool.tile([P, NP], BF, name=f"frT{k}") for k in range(KC)]
            WwT_sb = [cpool.tile([P, D], BF, name=f"WwT{k}") for k in range(KC)]
            Wb_sb = [cpool.tile([P, 1], DT, name=f"Wb{k}") for k in range(KC)]
            wext_sb = [cpool.tile([P, 2 * P], BF, name=f"wext{k}") for k in range(KC)]
            for k in range(KC):
                nc.sync.dma_start(out=frT_sb[k][:, :], in_=frT[k * P : (k + 1) * P, :])
                nc.gpsimd.dma_start(
                    out=WwT_sb[k][:, :], in_=WwT[k * P : (k + 1) * P, :]
                )
            nc.sync.dma_start(out=Wb_sb[0][:, :], in_=Wb[0:P, :])
            nc.gpsimd.dma_start(out=Wb_sb[1][:, :], in_=Wb[P : 2 * P, :])
            for k in range(KC):
                nc.sync.dma_start(out=WpT_sb[k][:, :], in_=WpT[k * P : (k + 1) * P, :])
                nc.gpsimd.dma_start(
                    out=fpT_sb[k][:, :], in_=fpT[k * P : (k + 1) * P, :]
                )
                nc.sync.dma_start(out=Wpb_sb[k][:, :], in_=Wpb[k * P : (k + 1) * P, :])
                nc.gpsimd.dma_start(
                    out=wext_sb[k][:, :], in_=wext[k * P : (k + 1) * P, :]
                )
            fp_sb = []
            for j in range(M // P):
                t_fp = cpool.tile([P, D], DT, name=f"fp{j}")
                (nc.sync if j % 2 else nc.gpsimd).dma_start(
                    out=t_fp[:, :], in_=fp[j * P : (j + 1) * P, :]
                )
                fp_sb.append(t_fp)
            wpB_sb = cpool.tile([P, D], DT, name="wpB")
            nc.sync.dma_start(out=wpB_sb[:, :], in_=wpB[:, :])
            ident_sb = cpool.tile([P, P], DT, name="ident")
            nc.gpsimd.dma_start(out=ident_sb[:, :], in_=ident[:, :])

            # ---- kT[d, m] = Wp_w @ f_r_prime^T (+Wpb); qT[d, n] likewise.
            # Chunk-0 of both is computed first so phase B group 0 unblocks
            # as early as possible.
            kT_sb = [cpool.tile([P, M], DT, name=f"kT{i}") for i in range(KC)]
            qT_sb = [cpool.tile([P, NP], DT, name=f"qT{i}") for i in range(KC)]

            def prep_chunk(i):
                q_ps = ps_qk.tile([P, 512], DT, name="q_ps", tag="qk")
                for k in range(KC):
                    nc.tensor.matmul(
                        q_ps[:, :NP],
                        lhsT=WwT_sb[k][:, i * P : (i + 1) * P],
                        rhs=frT_sb[k][:, :],
                        start=(k == 0),
                        stop=(k == KC - 1),
                    )
                nc.vector.tensor_scalar_add(
                    qT_sb[i][:, :], q_ps[:, :NP], Wb_sb[i][:, 0:1]
                )
                for h in range(M // 512):
                    k_ps = ps_qk.tile([P, 512], DT, name="k_ps", tag="qk")
                    for k in range(KC):
                        nc.tensor.matmul(
                            k_ps[:, :],
                            lhsT=WpT_sb[k][:, i * P : (i + 1) * P],
                            rhs=fpT_sb[k][:, h * 512 : (h + 1) * 512],
                            start=(k == 0),
                            stop=(k == KC - 1),
                        )
                    nc.vector.tensor_scalar_add(
                        kT_sb[i][:, h * 512 : (h + 1) * 512],
                        k_ps[:, :],
                        Wpb_sb[i][:, 0:1],
                    )

            prep_chunk(0)

            # ---- phase B: S[n, m] = sum_d w[d] * tanh(kT[d, m] + qT[d, n]) ----
            # (n, chunk) pairs are processed in groups of B; the per-row bias
            # add runs on DVE (tensor_scalar, 2x fp32), tanh runs on ScalarE
            # over the whole [P, B*M] group tile (amortizes the per-instr
            # overhead), and the PE reduces with bf16 inputs at full rate.
            BMAX = 8  # max (n, chunk) pairs per ACT instruction
            NITEMS = NP * KC
            # chunk-0 items first (group 0 only needs kT[0]); last groups
            # smaller to shorten the serial PE drain after the final tanh.
            items = [(n, 0) for n in range(NP)] + [(n, 1) for n in range(NP)]
            group_sizes = [2, 2, 4, 4] + [8] * 29 + [4, 4, 2, 2]
            assert sum(group_sizes) == NITEMS
            S_ps = ps_s.tile([P, M], DT, name="S_ps")
            pos = 0
            for gi, B in enumerate(group_sizes):
                if gi == 4:
                    prep_chunk(1)
                batch = items[pos : pos + B]
                first, last = pos == 0, pos + B == NITEMS
                pos += B
                Tin = wpool.tile([P, BMAX * M], BF, name="Tin", bufs=2)
                for j, (n, ci) in enumerate(batch):
                    nc.vector.tensor_scalar_add(
                        Tin[:, j * M : (j + 1) * M],
                        kT_sb[ci][:, :],
                        qT_sb[ci][:, n : n + 1],
                    )
                Tout = wpool.tile([P, BMAX * M], BF, name="Tout", bufs=2)
                nc.scalar.activation(
                    Tout[:, : B * M], Tin[:, : B * M],
                    mybir.ActivationFunctionType.Tanh,
                )
                for j, (n, ci) in enumerate(batch):
                    for h in range(M // 512):
                        nc.tensor.matmul(
                            S_ps[:, h * 512 : (h + 1) * 512],
                            lhsT=wext_sb[ci][:, P - n : 2 * P - n],
                            rhs=Tout[:, j * M + h * 512 : j * M + (h + 1) * 512],
                            start=(first and j == 0),
                            stop=(last and j == B - 1),
                        )

            # ---- softmax over m (unnormalized; row scale applied to context) ----
            # |S| <= sum|w| ~ 8, so exp(S) is fp32-safe without the usual
            # max-subtraction (softmax is shift-invariant); two half-width
            # instructions let the first transposes start earlier.
            expS = wpool.tile([P, M], DT, name="expS", bufs=1)
            sumex = spool.tile([P, 2], DT, name="sumex")
            for h in range(2):
                nc.scalar.activation(
                    expS[:, h * 512 : (h + 1) * 512],
                    S_ps[:, h * 512 : (h + 1) * 512],
                    mybir.ActivationFunctionType.Exp,
                    accum_out=sumex[:, h : h + 1],
                )
            sumt = spool.tile([P, 1], DT, name="sumt")
            nc.vector.tensor_add(sumt[:, :], sumex[:, 0:1], sumex[:, 1:2])
            rs = spool.tile([P, 1], DT, name="rs")
            nc.vector.reciprocal(rs[:, :], sumt[:, :])

            # ---- context = alpha @ f_r_prime ----
            ctx_ps = ps_ctx.tile([P, D], DT, name="ctx_ps")
            for j in range(M // P):
                tr_ps = ps_tr.tile([P, P], DT, name="tr_ps")
                nc.tensor.transpose(
                    tr_ps[:, :], expS[:, j * P : (j + 1) * P], ident_sb[:, :]
                )
                aT = apool.tile([P, P], DT, name="aT")
                nc.vector.tensor_copy(aT[:, :], tr_ps[:, :])
                nc.tensor.matmul(
                    ctx_ps[:, :],
                    lhsT=aT[:, :],
                    rhs=fp_sb[j][:, :],
                    start=(j == 0),
                    stop=(j == M // P - 1),
                )
            ctx_sb = wpool.tile([P, D], DT, name="ctx_sb", bufs=1)
            nc.vector.tensor_scalar_mul(ctx_sb[:, :], ctx_ps[:, :], rs[:, 0:1])

            # ---- per-row pooling score s[n] = context[n, :] . wp_w ----
            tmp = wpool.tile([P, D], DT, name="tmp", bufs=1)
            nc.vector.tensor_mul(tmp[:, :], ctx_sb[:, :], wpB_sb[:, :])
            s_sb = spool.tile([P, 1], DT, name="s_sb")
            nc.vector.reduce_sum(s_sb[:, :], tmp[:, :], axis=mybir.AxisListType.X)

            # ---- outputs ----
            nc.sync.dma_start(out=ctx_out[:, :], in_=ctx_sb[:, :])
            nc.sync.dma_start(out=s_out[:, :], in_=s_sb[:, :])

    nc.finalize()
    return nc


def _prep_inputs(f_r, f_r_prime, W_w, W_b, Wp_w, Wp_b, w_w, w_b, wp_w, wp_b):
    """Host-side layout prep (transposes / broadcasts only) + sharding."""
    import ml_dtypes

    BF_NP = ml_dtypes.bfloat16
    fpT = np.ascontiguousarray(f_r_prime.T).astype(BF_NP)
    fp = np.ascontiguousarray(f_r_prime, dtype=F32)
    WwT = np.ascontiguousarray(W_w.T).astype(BF_NP)
    WpT = np.ascontiguousarray(Wp_w.T).astype(BF_NP)
    Wb = np.ascontiguousarray(W_b.reshape(D, 1), dtype=F32)
    Wpb = np.ascontiguousarray(Wp_b.reshape(D, 1), dtype=F32)
    # zero-embedded w: column P (=128) holds w, so slice [P-n : 2P-n] puts w
    # in column n of the stationary operand.
    wext = np.zeros((D, 2 * P), dtype=F32)
    wext[:, P] = w_w.reshape(D)
    wext = wext.astype(BF_NP)
    wpB = np.broadcast_to(wp_w.reshape(1, D), (P, D)).copy()
    ident = np.eye(P, dtype=F32)

    shared = {
        "fpT": fpT,
        "fp": fp,
        "WwT": WwT,
        "WpT": WpT,
        "Wb": Wb,
        "Wpb": Wpb,
        "wext": wext,
        "wpB": wpB,
        "ident": ident,
    }
    in_maps = []
    for c in range(N_CORES):
        frT = np.ascontiguousarray(f_r[c * NP : (c + 1) * NP, :].T).astype(BF_NP)
        in_maps.append({"frT": frT, **shared})
    return in_maps


def _run(in_maps, **kw):
    if "nc" not in _CACHE:
        _CACHE["nc"] = build_nc()
    return run_bass_kernel_spmd(_CACHE["nc"], in_maps, list(range(N_CORES)), **kw)


def kernel(f_r, f_r_prime, W_w, W_b, Wp_w, Wp_b, w_w, w_b, wp_w, wp_b):
    in_maps = _prep_inputs(
        f_r, f_r_prime, W_w, W_b, Wp_w, Wp_b, w_w, w_b, wp_w, wp_b
    )
    res = _run(in_maps)
    ctx = np.concatenate([res.results[c]["ctx_out"] for c in range(N_CORES)], axis=0)
    s = np.concatenate(
        [res.results[c]["s_out"][:, 0] for c in range(N_CORES)], axis=0
    ).astype(np.float64)
    # final cross-shard softmax over N + pooled sum (the "all-reduce" step)
    s -= s.max()
    e = np.exp(s)
    a = (e / e.sum()).astype(F32)
    pool = a[None, :] @ ctx  # [1, D]
    return pool.astype(F32)
